# revision 17
# baseline (speedup 1.0000x reference)
"""NuFFT forward (KbNufft-style) Trainium2 Bass kernel.

Strategy:
  - Visibilities only touch |k| <= ~400 of the 2048-point oversampled grid
    (UMAX bound), so each of the 8 cores computes a 105-row x 804-col slab of
    the spectrum via DFT matmuls (apodization folded into the DFT constants):
        slab = Fv_rows . (cube/apod) . Fu_cols^T
  - Visibilities are sharded across cores by their v-row bin, so every
    core's slab fully covers its own visibilities' 6x6 KB footprints.
  - The slab is stored channel-interleaved in DRAM (row = [col][chan][re/im],
    padded to a 256B-multiple row stride); the 6x6 interpolation becomes bulk
    gpsimd.dma_gather calls (256B descriptors; visibilities binned by
    col-offset residue j0%8 so int16 indices address 64-f32-aligned starts
    from an 8*r f32 base offset), then a DVE multiply-reduce against
    host-precomputed 48-tap weight products (6 rows x 8 cols, last 2 zero).
"""
import os
import sys

for _p in ("/opt/trn_rl_repo",):
    if _p not in sys.path and os.path.isdir(_p):
        sys.path.insert(0, _p)

import numpy as np

# ---- problem constants (must match reference.py) ----
NCH = 4
NPIX = 1024
NVIS = 200_000
G = 2048
J = 6
OSF = 2
CELL_ARCSEC = 0.005
DL = CELL_ARCSEC * np.pi / (180.0 * 3600.0)
BETA = float(np.pi * np.sqrt((J / OSF) ** 2 * (OSF - 0.5) ** 2 - 0.8))

# ---- sharding geometry ----
N_CORES = 8
P = 128                      # SBUF partitions
ROW_LO_ALL = -398            # min possible m0 (floor of t), |t| < 397.2
ROWS_PER_CORE = 100
R_ROWS = ROWS_PER_CORE + 5   # 105 slab rows per core (footprint halo)
KU = 804                     # slab cols, c'_u in [-401, 403)
COL_BASE = -401
ROW_F32 = 6464               # padded slab row: 808 cols * 8 = 101*256B stride
STRIP = 408                  # stage-2 ku strip width (2 overlapping strips)
SOFF = (0, 396)              # strip col offsets; windows never straddle
N1 = 3 * R_ROWS + 1          # stage-1 rhs width (f32r needs even N)

NRES = 8                     # col-residue streams per strip
R_SLOTS = 14                 # vis slots per partition per (strip, residue)
N_STREAMS = 2 * NRES         # 16 gather streams
V_SLOTS = N_STREAMS * R_SLOTS        # 224 output rows per partition
GSTRIDE = 3328               # f32 per strip-grid row (52*256B stride)
GBLK = GSTRIDE // 64         # 52 64-f32 blocks per row
DESC_PER_S = P * R_SLOTS * J         # 10752 descriptors per stream
CALL_IDX = 1024                      # dma_gather ring capacity per call
IDXCOLS_S = DESC_PER_S // 16          # 672 int16 cols per stream
GROWS2 = (R_ROWS * GSTRIDE - 56) // 64  # 5459 64-f32 rows addressable

C1 = np.float32(1000.0 * 2.0 * np.pi * DL)   # klambda -> rad/pixel
C2 = np.float32(G / (2.0 * np.pi))           # rad/pixel -> grid coord

_NC_CACHE = {}


def _matmul_dtype():
    return os.environ.get("NUFFT_MM_DTYPE", "float32r")


def build_nc():
    """Build the SPMD Bass program (same program for all 8 cores)."""
    key = _matmul_dtype()
    if key in _NC_CACHE:
        return _NC_CACHE[key]

    import concourse.bacc as bacc
    import concourse.mybir as mybir
    import concourse.tile as tile
    from concourse import library_config
    from contextlib import ExitStack

    f32 = mybir.dt.float32
    i16 = mybir.dt.int16
    mm_dt = getattr(mybir.dt, key)

    nc = bacc.Bacc("TRN2", target_bir_lowering=False, debug=False)

    cube_d = nc.dram_tensor("cube", (NCH, NPIX, NPIX), mm_dt, kind="ExternalInput")
    cvt_d = nc.dram_tensor("cvt", (P, 8, N1), mm_dt, kind="ExternalInput")
    cut_d = nc.dram_tensor("cut", (P, 8, KU), mm_dt, kind="ExternalInput")
    sut_d = nc.dram_tensor("sut", (P, 8, KU), mm_dt, kind="ExternalInput")
    gidx_d = nc.dram_tensor("gidx", (P, N_STREAMS * IDXCOLS_S), i16,
                            kind="ExternalInput")
    w48_d = nc.dram_tensor("w48", (P, V_SLOTS, 48), f32, kind="ExternalInput")
    out_d = nc.dram_tensor("vis_out", (P, V_SLOTS, 8), f32,
                           kind="ExternalOutput")
    grid_d = [nc.dram_tensor(f"gridscratch{i}", (R_ROWS, GSTRIDE), f32)
              for i in range(2)]


    with tile.TileContext(nc) as tc:
        with ExitStack() as s12:
            const_pool = s12.enter_context(tc.tile_pool(name="const", bufs=1))
            cube_pool = s12.enter_context(tc.tile_pool(name="cube", bufs=3))
            tpool = s12.enter_context(tc.tile_pool(name="tmats", bufs=1))
            cpool = s12.enter_context(tc.tile_pool(name="cstream", bufs=4))
            psum_pool = s12.enter_context(
                tc.tile_pool(name="ps", bufs=8, space="PSUM"))

            cvt_sb = const_pool.tile([P, 8, N1], mm_dt)
            nc.sync.dma_start(cvt_sb[:], cvt_d[:])

            # T storage: (p, chan, term[T1,T2,negT1], xc, r)
            tall = tpool.tile([P, NCH, 3, 8, R_ROWS], mm_dt)

            # ---- stage 1: T^T = cube^T . cvt (accumulate over y chunks) ----
            for c in range(NCH):
                ps = [psum_pool.tile([P, N1], f32, tag="ps",
                                     name=f"ps1_{c}_{i}") for i in range(8)]
                for yc in range(8):
                    cb = cube_pool.tile([P, NPIX], mm_dt, tag="cube")
                    nc.sync.dma_start(cb[:], cube_d[c, yc * P:(yc + 1) * P, :])
                    for xt in range(8):
                        nc.tensor.matmul(
                            ps[xt][:],
                            lhsT=cb[:, xt * P:(xt + 1) * P],
                            rhs=cvt_sb[:, yc, :],
                            start=(yc == 0),
                            stop=(yc == 7),
                        )
                for xt in range(8):
                    for term in range(3):
                        nc.vector.tensor_copy(
                            tall[:, c, term, xt, :],
                            ps[xt][:, term * R_ROWS:(term + 1) * R_ROWS],
                        )

            # ---- stage 2: slab = T . [cut|sut], interleave, DMA to DRAM ----
            grid_sb = tpool.tile([P, KU * 8], f32)
            gv = grid_sb[:].rearrange("p (col e) -> p col e", e=8)
            zpad = cpool.tile([P, GSTRIDE - STRIP * 8], f32, tag="zpad")
            nc.gpsimd.memset(zpad[:], 0.0)
            for strip in range(2):
                off = SOFF[strip]
                ps2 = [psum_pool.tile([P, STRIP], f32, tag="ps",
                                      name=f"ps2_{strip}_{i}")
                       for i in range(8)]  # (c, re/im) -> ps2[c*2+e]
                for xc in range(8):
                    cu = cpool.tile([P, STRIP], mm_dt, tag="cu")
                    nc.sync.dma_start(
                        cu[:], cut_d[:, xc, off:off + STRIP])
                    su = cpool.tile([P, STRIP], mm_dt, tag="su")
                    nc.sync.dma_start(
                        su[:], sut_d[:, xc, off:off + STRIP])
                    for c in range(NCH):
                        t1 = tall[:, c, 0, xc, :]
                        t2 = tall[:, c, 1, xc, :]
                        nt1 = tall[:, c, 2, xc, :]
                        cuv = cu[:]
                        suv = su[:]
                        # re = T1.cu + T2.su ; im = T2.cu + (-T1).su
                        nc.tensor.matmul(ps2[c * 2][:R_ROWS, :], lhsT=t1,
                                         rhs=cuv, start=(xc == 0), stop=False)
                        nc.tensor.matmul(ps2[c * 2][:R_ROWS, :], lhsT=t2,
                                         rhs=suv, start=False, stop=(xc == 7))
                        nc.tensor.matmul(ps2[c * 2 + 1][:R_ROWS, :], lhsT=t2,
                                         rhs=cuv, start=(xc == 0), stop=False)
                        nc.tensor.matmul(ps2[c * 2 + 1][:R_ROWS, :], lhsT=nt1,
                                         rhs=suv, start=False, stop=(xc == 7))
                skip = 0 if strip == 0 else (SOFF[0] + STRIP) - SOFF[1]
                for c in range(NCH):
                    for e in range(2):
                        nc.vector.tensor_copy(
                            gv[:R_ROWS, off + skip:off + STRIP, c * 2 + e],
                            ps2[c * 2 + e][:R_ROWS, skip:],
                        )
                # ship this strip's slab so its gathers can start early
                nc.sync.dma_start(
                    grid_d[strip][:, :STRIP * 8],
                    grid_sb[:R_ROWS, off * 8:(off + STRIP) * 8])
                nc.sync.dma_start(grid_d[strip][:, STRIP * 8:],
                                  zpad[:R_ROWS, :])

        # ---- stage 3: residue-binned dma_gather + weighted reduce ----
        with ExitStack() as s3:
            ipool = s3.enter_context(tc.tile_pool(name="interp", bufs=3))
            opool = s3.enter_context(tc.tile_pool(name="outp", bufs=1))

            nc.gpsimd.load_library(library_config.mlp)
            ov = opool.tile([P, V_SLOTS, 8], f32)
            flats = [grid_d[i][:, :].flatten() for i in range(2)]
            for st in range(N_STREAMS):
                sgrid, r = st // NRES, st % NRES
                view_r = flats[sgrid][8 * r: 8 * r + GROWS2 * 64].rearrange(
                    "(n e) -> n e", e=64)
                idxr = ipool.tile([P, IDXCOLS_S], i16, tag="idx",
                                  name=f"idx_{st}")
                nc.sync.dma_start(
                    idxr[:], gidx_d[:, st * IDXCOLS_S:(st + 1) * IDXCOLS_S])
                w = ipool.tile([P, R_SLOTS * 48], f32, tag="w", name=f"w_{st}")
                nc.sync.dma_start(
                    w[:],
                    w48_d[:, st * R_SLOTS:(st + 1) * R_SLOTS, :].rearrange(
                        "p v t -> p (v t)"))
                g = ipool.tile([P, R_SLOTS * J, 64], f32, tag="g",
                               name=f"g_{st}")
                done = 0
                k = 0
                while done < DESC_PER_S:
                    n_idx = min(CALL_IDX, DESC_PER_S - done)
                    nc.gpsimd.dma_gather(
                        out_ap=g[:, done // P:(done + n_idx) // P, :],
                        in_ap=view_r,
                        idxs_ap=idxr[:, done // 16:(done + n_idx) // 16],
                        num_idxs=n_idx,
                        num_idxs_reg=n_idx,
                        elem_size=64,
                        elem_step=64,
                    )
                    done += n_idx
                    k += 1
                # multiply by weights (broadcast over chan/reim)
                gw = g[:].rearrange("p t (col e) -> p (t col) e", e=8)
                wb = w[:].unsqueeze(2).to_broadcast([P, R_SLOTS * 48, 8])
                nc.vector.tensor_tensor(
                    out=gw, in0=gw, in1=wb, op=mybir.AluOpType.mult)
                # reduce over the 48 (6 rows x 8 cols, 2 zero) taps
                rv = g[:].rearrange(
                    "p (v i) (col e) -> p v e (i col)", v=R_SLOTS, i=J, e=8)
                nc.vector.tensor_reduce(
                    out=ov[:, st * R_SLOTS:(st + 1) * R_SLOTS, :],
                    in_=rv,
                    axis=mybir.AxisListType.X,
                    op=mybir.AluOpType.add,
                )
            nc.sync.dma_start(out_d[:], ov[:])

    nc.compile()
    _NC_CACHE[key] = nc
    return nc


def _apod1d():
    f = np.arange(NPIX, dtype=np.float64) / G
    z = np.pi * J * f
    s = np.sqrt(BETA * BETA - z * z)
    return J * np.sinh(s) / s  # [NPIX] float64


def _interp_host(k):
    """Match reference _interp_coords index/weight math in f32."""
    t = (k.astype(np.float32) * C1) * C2
    m0 = np.floor(t).astype(np.int32)
    offs = np.arange(J, dtype=np.int32) - (J // 2 - 1)
    d = t[:, None] - (m0[:, None] + offs).astype(np.float32)
    w = np.i0(BETA * np.sqrt(np.maximum(0.0, 1.0 - (2.0 * d / J) ** 2)))
    return t, m0, w.astype(np.float32)


def host_prep(cube, uu, vv):
    """Returns (in_maps, meta, phase) for the 8 cores."""
    mmkey = _matmul_dtype()
    if mmkey == "bfloat16":
        import ml_dtypes
        mmnp = ml_dtypes.bfloat16
    else:
        mmnp = np.float32
    cube = np.ascontiguousarray(np.asarray(cube, dtype=np.float32)).astype(mmnp)
    uu = np.asarray(uu, dtype=np.float32)
    vv = np.asarray(vv, dtype=np.float32)

    s1 = _apod1d()
    y = np.arange(NPIX, dtype=np.float64)

    # u-direction DFT constants (same for all cores)
    kj = np.arange(KU, dtype=np.float64) + COL_BASE
    ang_u = 2.0 * np.pi * np.outer(y, kj) / G
    cut = (np.cos(ang_u) / s1[:, None]).astype(np.float32)
    sut = (np.sin(ang_u) / s1[:, None]).astype(np.float32)
    cut = np.ascontiguousarray(cut.reshape(8, P, KU).transpose(1, 0, 2)).astype(mmnp)
    sut = np.ascontiguousarray(sut.reshape(8, P, KU).transpose(1, 0, 2)).astype(mmnp)

    tu, m0u, wu = _interp_host(uu)
    tv, m0v, wv = _interp_host(vv)
    assert m0u.min() >= ROW_LO_ALL and m0u.max() < ROW_LO_ALL + 8 * ROWS_PER_CORE
    assert m0v.min() >= ROW_LO_ALL and m0v.max() < ROW_LO_ALL + 8 * ROWS_PER_CORE

    core_of = (m0v - ROW_LO_ALL) // ROWS_PER_CORE
    j0 = m0u - 2 - COL_BASE        # window start col within slab, [1, 796]
    sgrid = (j0 > 400).astype(np.int64)
    colp = j0 - 396 * sgrid        # col within strip grid, [1,400] or [5,407]
    res = colp % NRES
    q = colp // NRES               # 64-f32 block within strip row, [0, 50]
    w48 = np.zeros((len(uu), J, 8), dtype=np.float32)
    w48[:, :, :J] = wv[:, :, None] * wu[:, None, :]

    in_maps = []
    meta = []
    for k in range(N_CORES):
        row_lo = ROW_LO_ALL + ROWS_PER_CORE * k
        gidx = np.zeros((P, N_STREAMS * IDXCOLS_S), dtype=np.int16)
        w48k = np.zeros((P, V_SLOTS, 48), dtype=np.float32)
        meta_k = []
        for st in range(N_STREAMS):
            sg, r = st // NRES, st % NRES
            order = np.where((core_of == k) & (sgrid == sg) & (res == r))[0]
            n = len(order)
            assert n <= P * R_SLOTS, f"core {k} stream {st} overflow: {n}"
            sl = np.arange(n)
            pp = sl % P
            vs = sl // P
            lrow = (m0v[order] - row_lo).astype(np.int64)   # [0, 100)
            vals = (lrow[:, None] + np.arange(J)[None, :]) * GBLK \
                + q[order, None].astype(np.int64)           # [n, J] <= 5458
            # descriptor t = (v*6+i)*128 + p ; idx A[t%16, t//16]
            t = (vs[:, None] * J + np.arange(J)[None, :]) * P + pp[:, None]
            block = np.zeros((16, IDXCOLS_S), dtype=np.int16)
            block[(t % 16).ravel(), (t // 16).ravel()] = vals.astype(
                np.int16).ravel()
            gidx[:, st * IDXCOLS_S:(st + 1) * IDXCOLS_S] = np.tile(block,
                                                                   (8, 1))
            w48k[pp, st * R_SLOTS + vs, :] = w48[order].reshape(n, 48)
            meta_k.append((order, pp, st * R_SLOTS + vs))
        # v-direction DFT constants for this core's rows
        kr = np.arange(R_ROWS, dtype=np.float64) + (row_lo - 2)
        ang_v = 2.0 * np.pi * np.outer(y, kr) / G
        blk = np.zeros((NPIX, 3 * R_ROWS + 1), dtype=np.float32)
        cosb = np.cos(ang_v) / s1[:, None]
        sinb = np.sin(ang_v) / s1[:, None]
        blk[:, 0 * R_ROWS:1 * R_ROWS] = cosb
        blk[:, 1 * R_ROWS:2 * R_ROWS] = -sinb
        blk[:, 2 * R_ROWS:3 * R_ROWS] = -cosb
        cvt = np.ascontiguousarray(
            blk.reshape(8, P, 3 * R_ROWS + 1).transpose(1, 0, 2)).astype(mmnp)

        in_maps.append({
            "cube": cube,
            "cvt": cvt,
            "cut": cut,
            "sut": sut,
            "gidx": gidx,
            "w48": w48k,
        })
        meta.append(meta_k)

    kv = vv * C1
    ku_ = uu * C1
    phase = np.exp(1j * (kv + ku_) * np.float32(NPIX / 2.0)).astype(np.complex64)
    return in_maps, meta, phase


def assemble(results, meta, phase):
    out = np.zeros((NCH, NVIS), dtype=np.complex64)
    for k in range(N_CORES):
        arr = results[k]["vis_out"].reshape(P, V_SLOTS, NCH, 2)
        for order, pp, rows in meta[k]:
            vals = arr[pp, rows]  # [n, NCH, 2]
            out[:, order] = (vals[..., 0] + 1j * vals[..., 1]).T
    return out * phase[None, :]


def kernel(cube, uu, vv):
    from concourse.bass_utils import run_bass_kernel_spmd

    nc = build_nc()
    in_maps, meta, phase = host_prep(cube, uu, vv)
    br = run_bass_kernel_spmd(
        nc, in_maps, list(range(N_CORES)),
        trace=bool(int(os.environ.get("NUFFT_TRACE", "0"))),
    )
    if br.exec_time_ns is not None:
        print(f"HW exec time: {br.exec_time_ns} ns")
    kernel.last_result = br
    return assemble(br.results, meta, phase)
